# revision 24
# baseline (speedup 1.0000x reference)
"""BertSelfAttention Trainium2 kernel.

Shapes: hidden_states [S=1024, B=4, D=1024], H=16 heads of DH=64.
Sharding: 2 heads per core (8 cores). Each core receives the full hidden
states (pre-transposed + bf16-cast on host) and a 128-row slice of each
projection weight, computes the full attention chain for its two heads with
no cross-core communication, and writes ctx^T per (batch, head).

Device-side layout:
  - scores are computed transposed (scoresT[u, q] = q_q . k_u) so the
    additive attention mask (per key position u) is a per-partition bias
    that fuses into the Exp activation: probsT = exp(scores/8 + mask).
  - the two heads' score matmuls have contraction 64, so they land on PE
    row-tiles (0,0)/(64,0) and run concurrently; each (uc, head) score
    tile is [128, 1024] spanning 2 PSUM banks so a single N=1024 Exp
    serves both q-chunks.
  - V carries a prepended ones-column, so the AV matmul produces the
    softmax denominator in row 0 of ctxT for free; normalization is
    reciprocal_approx_fast + gpsimd partition_broadcast + one multiply,
    done per 512-wide chunk as soon as its ctx accumulation finishes.
  - AV of batch b-1 is woven chunk-serial through batch b's uc loop
    (h0c0 in uc0-1, h0c1 in uc2-3, h1c0 in uc4-5, h1c1 in uc6-7) so only
    one ctx PSUM bank is held at a time; batch 3's own h0 AV is woven
    into its loop and only h1 remains in the epilogue.
"""

import os
import numpy as np
import ml_dtypes

S, B, D, H = 1024, 4, 1024, 16
DH = D // H          # 64
NCORES = 8
HPC = H // NCORES    # heads per core = 2
P = 128              # partitions / d-tile / u-tile
DCH = D // P         # 8 contraction tiles
BS = B * S           # 4096 flattened (b, s)
CH = 512             # matmul free-dim chunk (PSUM bank limit)

_compiled_nc = None
last_exec_time_ns = None
last_results = None


def _build():
    import concourse.bacc as bacc
    import concourse.mybir as mybir
    import concourse.tile as tile
    from contextlib import ExitStack

    f32 = mybir.dt.float32
    bf16 = mybir.dt.bfloat16
    AF = mybir.ActivationFunctionType

    nc = bacc.Bacc("TRN2", target_bir_lowering=False, debug=False,
                   num_devices=NCORES)

    # hT host-packed as [p, b, dc, t'] so each partition's per-batch slice
    # is 16KB contiguous (big DMA descriptors -> near-peak DMA bandwidth).
    hT_d = nc.dram_tensor("hT", [P, B * DCH * S], bf16, kind="ExternalInput")
    # weights host-packed to the SBUF layout [p, dc*P+m] so the upload DMA
    # reads 2KB contiguous lines per partition.
    wqT_d = nc.dram_tensor("wqT", [P, DCH * P], bf16, kind="ExternalInput")
    wkT_d = nc.dram_tensor("wkT", [P, DCH * P], bf16, kind="ExternalInput")
    wvT_d = nc.dram_tensor("wvT", [P, DCH * P], bf16, kind="ExternalInput")
    # packed per-partition constants: [bq | bk | bvb(128) | maskT(8*4)]
    misc_d = nc.dram_tensor("misc", [P, 2 + P + DCH * B], f32,
                            kind="ExternalInput")
    out_d = nc.dram_tensor("out", [B, HPC, DH, S], f32, kind="ExternalOutput")

    with tile.TileContext(nc) as tc, ExitStack() as ctx:
        persist = ctx.enter_context(tc.tile_pool(name="persist", bufs=1))
        pp_pool = ctx.enter_context(tc.tile_pool(name="pp", bufs=34))
        stage = ctx.enter_context(tc.tile_pool(name="stage", bufs=8))
        ps_sc = ctx.enter_context(tc.tile_pool(name="ps_sc", bufs=2, space="PSUM"))
        ps_mm = ctx.enter_context(tc.tile_pool(name="ps_mm", bufs=2, space="PSUM"))
        ps_ctx = ctx.enter_context(tc.tile_pool(name="ps_ctx", bufs=2, space="PSUM"))

        # ---- persistent SBUF tensors ----
        hT_sb = persist.tile([P, B, DCH, S], bf16)      # hidden^T, batch/d-tiled
        wq_sb = persist.tile([P, DCH, P], bf16)
        wk_sb = persist.tile([P, DCH, P], bf16)
        wv_sb = persist.tile([P, DCH, P], bf16)
        misc_sb = persist.tile([P, 2 + P + DCH * B], f32)
        qT_sb = persist.tile([P, BS], bf16)             # Q^T [i, q]
        kT_sb = persist.tile([P, BS], bf16)             # K^T [i, u]
        # V in [u, j] layout + ones column per head: [u-part, u-tile, head, DH+1]
        v_sb = persist.tile([P, BS // P, HPC, DH + 1], bf16)
        dummy_sb = persist.tile([P, P], bf16)

        bq_sb = misc_sb[:, 0:1]
        bk_sb = misc_sb[:, 1:2]
        bvb_sb = misc_sb[:, 2:2 + P]

        def mask_bias(uc, bi):
            c = 2 + P + uc * B + bi
            return misc_sb[:, c:c + 1]

        # ---- HAM warmup: dead matmuls keep the PE busy while inputs load,
        # so the real work starts at the 2.4 GHz clock.
        nc.vector.memset(dummy_sb[:], 0.0)
        d_ps = ps_mm.tile([P, P], f32, tag="mm", name="d_ps")
        for _ in range(48):
            nc.tensor.matmul(d_ps[:], dummy_sb[:], dummy_sb[:],
                             start=True, stop=True)

        # ---- input DMAs ----
        # Queue plan: weights+misc ride the gpsimd queue (idle early);
        # batch 0 lands as two partition-half DMAs on sync/scalar (16KB
        # contiguous per partition -> near-peak bandwidth); batches 1-3 as
        # one DMA each on sync/gpsimd, keeping the scalar queue free for
        # activations (a queued DMA issue blocks the queue ~2us on
        # semaphore recycling once >4 are in flight).
        hT_re = hT_d.ap().rearrange("p (b dc t) -> p b dc t", dc=DCH, t=S)

        nc.gpsimd.dma_start(wq_sb[:],
                            wqT_d.ap().rearrange("p (dc m) -> p dc m", m=P))
        nc.gpsimd.dma_start(wk_sb[:],
                            wkT_d.ap().rearrange("p (dc m) -> p dc m", m=P))
        nc.sync.dma_start(hT_sb[0:64, 0, :, :], hT_re[0:64, 0, :, :])
        nc.scalar.dma_start(hT_sb[64:P, 0, :, :], hT_re[64:P, 0, :, :])
        nc.gpsimd.dma_start(wv_sb[:],
                            wvT_d.ap().rearrange("p (dc m) -> p dc m", m=P))
        nc.gpsimd.dma_start(misc_sb[:], misc_d.ap())
        for bslot, eng in ((1, nc.sync), (2, nc.gpsimd), (3, nc.sync)):
            eng.dma_start(hT_sb[:, bslot, :, :], hT_re[:, bslot, :, :])

        nc.vector.memset(v_sb[:, :, :, 0:1], 1.0)

        scale = 1.0 / float(np.sqrt(DH))

        # ---- projection thunks -------------------------------------------
        def emit_qk_chunk(w_sb, b_sb, dst, ci):
            sl = slice(ci * CH, (ci + 1) * CH)
            lsl = slice((ci % 2) * CH, (ci % 2 + 1) * CH)
            qk_ps = ps_mm.tile([P, CH], f32, tag="mm", name="qk_ps")
            for dc in range(DCH):
                nc.tensor.matmul(
                    qk_ps[:], w_sb[:, dc, :], hT_sb[:, ci // 2, dc, lsl],
                    start=(dc == 0), stop=(dc == DCH - 1))
            nc.vector.tensor_scalar_add(dst[:, sl], qk_ps[:], b_sb[:])

        def emit_v_tile(tt):
            tsl = slice((tt % DCH) * P, (tt % DCH + 1) * P)
            v_ps = ps_mm.tile([P, P], f32, tag="mm", name="v_ps")
            for dc in range(DCH):
                nc.tensor.matmul(
                    v_ps[:], hT_sb[:, tt // DCH, dc, tsl], wv_sb[:, dc, :],
                    start=(dc == 0), stop=(dc == DCH - 1))
            nc.vector.tensor_add(
                v_sb[:, tt, 0:HPC, 1:DH + 1],
                v_ps[:].rearrange("p (h j) -> p h j", j=DH),
                bvb_sb[:].rearrange("p (h j) -> p h j", j=DH))

        def qk_thunks(bi):
            th = []
            for w_sb, b_sb, dst in ((wq_sb, bq_sb, qT_sb), (wk_sb, bk_sb, kT_sb)):
                for ci in range(2 * bi, 2 * bi + 2):
                    th.append((4096, lambda w=w_sb, b=b_sb, d=dst, c=ci:
                               emit_qk_chunk(w, b, d, c)))
            return th

        def v_thunks(bi):
            return [(1024, lambda t=tt: emit_v_tile(t))
                    for tt in range(8 * bi, 8 * bi + 8)]

        # ---- AV + normalization ------------------------------------------
        def emit_norm(bi, hl, c2, ctx_ps):
            csl = slice(c2 * CH, (c2 + 1) * CH)
            rcp_sb = stage.tile([1, CH], f32, tag="rcp", bufs=2, name="rcp_sb")
            nc.vector.reciprocal_approx_fast(rcp_sb[:], ctx_ps[0:1, :])
            rcpb_sb = stage.tile([DH + 1, CH], f32, tag="rcpb", bufs=2,
                                 name="rcpb_sb")
            nc.gpsimd.partition_broadcast(rcpb_sb[:], rcp_sb[:])
            o_sb = stage.tile([DH + 1, CH], f32, tag="o", bufs=3, name="o_sb")
            nc.vector.tensor_mul(o_sb[:], ctx_ps[:], rcpb_sb[:])
            nc.sync.dma_start(out_d.ap()[bi, hl, :, csl], o_sb[1:DH + 1, :])

        def emit_av_mm(bi, hl, c2, ctx_ps, uc, pps):
            nc.tensor.matmul(
                ctx_ps[:],
                v_sb[:, bi * 8 + uc, hl, :],
                pps[(uc, hl)][:, c2 * CH:(c2 + 1) * CH],
                start=(uc == 0), stop=(uc == DCH - 1))

        # ---- prologue: batch 0's Q/K, dc-major so all four PSUM groups
        # chase the arriving hT pieces concurrently.
        pro_specs = [(wq_sb, bq_sb, qT_sb, 0), (wq_sb, bq_sb, qT_sb, 1),
                     (wk_sb, bk_sb, kT_sb, 0), (wk_sb, bk_sb, kT_sb, 1)]
        # groups 0-1 use the first CH columns of a [P, S] ps_sc tile (2 banks
        # each); groups 2-3 use [P, CH] ps_mm tiles.
        pro_tiles = [ps_sc.tile([P, S], f32, tag="sc", name="pro_ps"),
                     ps_sc.tile([P, S], f32, tag="sc", name="pro_ps"),
                     ps_mm.tile([P, CH], f32, tag="mm", name="pro_ps"),
                     ps_mm.tile([P, CH], f32, tag="mm", name="pro_ps")]
        pro_ps = [t[:, 0:CH] for t in pro_tiles]
        for dc in range(DCH):
            for g, (w_sb, b_sb, dst, ci) in enumerate(pro_specs):
                nc.tensor.matmul(
                    pro_ps[g], w_sb[:, dc, :],
                    hT_sb[:, 0, dc, ci * CH:(ci + 1) * CH],
                    start=(dc == 0), stop=(dc == DCH - 1))
        for g, (w_sb, b_sb, dst, ci) in enumerate(pro_specs):
            osl = slice(ci * CH, (ci + 1) * CH)
            nc.vector.tensor_scalar_add(dst[:, osl], pro_ps[g], b_sb[:])

        # ---- main software pipeline over batches -------------------------
        queue = []          # (cycles, thunk) list, paced per uc
        queue += v_thunks(0) + qk_thunks(1) + v_thunks(1)
        prev_pps = None     # probs tiles of batch bi-1
        ctx_cur = None
        ctx3 = None

        for bi in range(B):
            if bi == 1:
                queue += qk_thunks(2) + v_thunks(2)
            elif bi == 2:
                queue += qk_thunks(3) + v_thunks(3)
            total_c = sum(c for c, _ in queue)
            popped = [0]

            def pace(uc, _total=total_c, _popped=popped, _queue=queue):
                want = (_total * (uc + 1)) // DCH
                while _queue and _popped[0] < want:
                    cyc, th = _queue.pop(0)
                    _popped[0] += cyc
                    th()

            pps = {}
            for uc in range(DCH):
                usl = slice(bi * S + uc * P, bi * S + (uc + 1) * P)
                # scores: both heads' chunk-c matmuls adjacent so the PE
                # row-tiles (0,0)/(64,0) run them concurrently.
                sc = [ps_sc.tile([P, S], f32, tag="sc", name="sc_ps") for _ in range(HPC)]
                for c2 in range(2):
                    csl = slice(c2 * CH, (c2 + 1) * CH)
                    qsl = slice(bi * S + c2 * CH, bi * S + (c2 + 1) * CH)
                    for hl in range(HPC):
                        hsl = slice(hl * DH, (hl + 1) * DH)
                        nc.tensor.matmul(
                            sc[hl][:, csl],
                            kT_sb[hsl, usl], qT_sb[hsl, qsl],
                            start=True, stop=True)
                for hl in range(HPC):
                    pp = pp_pool.tile([P, S], bf16, tag="pp", name="pp")
                    nc.scalar.activation(
                        pp[:], sc[hl][:], AF.Exp,
                        bias=mask_bias(uc, bi), scale=scale)
                    pps[(uc, hl)] = pp

                # AV of batch bi-1, chunk-serial: one ctx bank at a time.
                if prev_pps is not None:
                    seg = uc // 2          # 0:h0c0 1:h0c1 2:h1c0 3:h1c1
                    hl, c2 = seg // 2, seg % 2
                    half = uc % 2
                    if half == 0:
                        ctx_cur = ps_ctx.tile([DH + 1, CH], f32, tag="ctx", name="ctx_ps")
                    for up in range(4 * half, 4 * half + 4):
                        emit_av_mm(bi - 1, hl, c2, ctx_cur, up, prev_pps)
                    if half == 1:
                        emit_norm(bi - 1, hl, c2, ctx_cur)

                # batch 3's own h0 AV woven into its loop (ps_mm is idle).
                if bi == B - 1:
                    if uc == 0:
                        ctx3 = [ps_mm.tile([DH + 1, CH], f32, tag="mm", name="ctx3")
                                for _ in range(2)]
                    for c2 in range(2):
                        emit_av_mm(bi, 0, c2, ctx3[c2], uc, pps)
                    if uc == DCH - 1:
                        for c2 in range(2):
                            emit_norm(bi, 0, c2, ctx3[c2])

                pace(uc)
            prev_pps = pps

        # ---- epilogue: batch 3's h1 --------------------------------------
        ctxE = [ps_ctx.tile([DH + 1, CH], f32, tag="ctx", name="ctxE") for _ in range(2)]
        for c2 in range(2):
            for uc in range(DCH):
                emit_av_mm(B - 1, 1, c2, ctxE[c2], uc, prev_pps)
            emit_norm(B - 1, 1, c2, ctxE[c2])

    nc.compile()
    return nc


def _get_nc():
    global _compiled_nc
    if _compiled_nc is None:
        _compiled_nc = _build()
    return _compiled_nc


def prepare_in_maps(hidden_states, attention_mask, Wq, bq, Wk, bk, Wv, bv):
    bf16 = ml_dtypes.bfloat16

    hs = np.asarray(hidden_states, dtype=np.float32)            # [S, B, D]
    # [p, b, dc, t']: hT[p, b, dc, t'] = hs[t', b, dc*128+p]
    hT = np.ascontiguousarray(
        hs.reshape(S, B, DCH, P).transpose(3, 1, 2, 0)
          .reshape(P, B * DCH * S)).astype(bf16)
    maskT = np.ascontiguousarray(
        np.asarray(attention_mask, dtype=np.float32).reshape(B, S).T)
    Wq = np.asarray(Wq, dtype=np.float32)
    Wk = np.asarray(Wk, dtype=np.float32)
    Wv = np.asarray(Wv, dtype=np.float32)
    bq = np.asarray(bq, dtype=np.float32)
    bk = np.asarray(bk, dtype=np.float32)
    bv = np.asarray(bv, dtype=np.float32)

    # maskT packed as [p, uc, b] -> [128, 32]
    mask_pk = maskT.reshape(DCH, P, B).transpose(1, 0, 2).reshape(P, DCH * B)
    in_maps = []
    for c in range(NCORES):
        sl = slice(P * c, P * (c + 1))
        misc = np.empty((P, 2 + P + DCH * B), dtype=np.float32)
        misc[:, 0] = bq[sl]
        misc[:, 1] = bk[sl]
        misc[:, 2:2 + P] = np.broadcast_to(bv[sl][None, :], (P, P))
        misc[:, 2 + P:] = mask_pk
        def pack_w(W):
            # [p, dc*128+m] = W[sl][m, dc*128+p] -> contiguous upload rows
            wT = W[sl, :].T.reshape(DCH, P, P)          # [dc, p, m]
            return np.ascontiguousarray(
                wT.transpose(1, 0, 2).reshape(P, DCH * P)).astype(bf16)

        in_maps.append({
            "hT": hT,
            "wqT": pack_w(Wq),
            "wkT": pack_w(Wk),
            "wvT": pack_w(Wv),
            "misc": misc,
        })
    return in_maps


def kernel(hidden_states, attention_mask, Wq, bq, Wk, bk, Wv, bv):
    global last_exec_time_ns, last_results
    from concourse.bass_utils import run_bass_kernel_spmd

    nc = _get_nc()
    in_maps = prepare_in_maps(hidden_states, attention_mask,
                              Wq, bq, Wk, bk, Wv, bv)

    trace = bool(int(os.environ.get("KERNEL_TRACE", "0")))
    tmpdir = os.environ.get("KERNEL_TRACE_DIR") or None
    res = run_bass_kernel_spmd(nc, in_maps, core_ids=list(range(NCORES)),
                               trace=trace, tmpdir=tmpdir)
    last_exec_time_ns = res.exec_time_ns
    last_results = res

    # gather: per-core out [B, HPC, DH, S] -> full [S, B, D]
    outs = np.stack([np.asarray(res.results[c]["out"]) for c in range(NCORES)],
                    axis=0)                                     # [C, B, HPC, DH, S]
    full = outs.transpose(4, 1, 0, 2, 3).reshape(S, B, D)       # s, b, (c, hl, j)
    return np.ascontiguousarray(full.astype(np.float32))


# revision 26
# speedup vs baseline: 1.0610x; 1.0610x over previous
"""BertSelfAttention Trainium2 kernel.

Shapes: hidden_states [S=1024, B=4, D=1024], H=16 heads of DH=64.
Sharding: 2 heads per core (8 cores). Each core receives the full hidden
states (pre-transposed + bf16-cast on host) and a 128-row slice of each
projection weight, computes the full attention chain for its two heads with
no cross-core communication, and writes ctx^T per (batch, head).

Device-side layout:
  - scores are computed transposed (scoresT[u, q] = q_q . k_u) so the
    additive attention mask (per key position u) is a per-partition bias
    that fuses into the Exp activation: probsT = exp(scores/8 + mask).
  - the two heads' score matmuls have contraction 64, so they land on PE
    row-tiles (0,0)/(64,0) and run concurrently; each (uc, head) score
    tile is [128, 1024] spanning 2 PSUM banks so a single N=1024 Exp
    serves both q-chunks.
  - V carries a prepended ones-column, so the AV matmul produces the
    softmax denominator in row 0 of ctxT for free; normalization is
    reciprocal_approx_fast + gpsimd partition_broadcast + one multiply,
    done per 512-wide chunk as soon as its ctx accumulation finishes.
  - AV of batch b-1 is woven chunk-serial through batch b's uc loop
    (h0c0 in uc0-1, h0c1 in uc2-3, h1c0 in uc4-5, h1c1 in uc6-7) so only
    one ctx PSUM bank is held at a time; batch 3's own h0 AV is woven
    into its loop and only h1 remains in the epilogue.
"""

import os
import numpy as np
import ml_dtypes

S, B, D, H = 1024, 4, 1024, 16
DH = D // H          # 64
NCORES = 8
HPC = H // NCORES    # heads per core = 2
P = 128              # partitions / d-tile / u-tile
DCH = D // P         # 8 contraction tiles
BS = B * S           # 4096 flattened (b, s)
CH = 512             # matmul free-dim chunk (PSUM bank limit)

_compiled_nc = None
last_exec_time_ns = None
last_results = None


def _build():
    import concourse.bacc as bacc
    import concourse.mybir as mybir
    import concourse.tile as tile
    from contextlib import ExitStack

    f32 = mybir.dt.float32
    bf16 = mybir.dt.bfloat16
    AF = mybir.ActivationFunctionType

    nc = bacc.Bacc("TRN2", target_bir_lowering=False, debug=False,
                   num_devices=NCORES)

    # hT host-packed as [p, b, dc, t'] so each partition's per-batch slice
    # is 16KB contiguous (big DMA descriptors -> near-peak DMA bandwidth).
    hT_d = nc.dram_tensor("hT", [P, B * DCH * S], bf16, kind="ExternalInput")
    # weights host-packed to the SBUF layout [p, dc*P+m] so the upload DMA
    # reads 2KB contiguous lines per partition.
    wqT_d = nc.dram_tensor("wqT", [P, DCH * P], bf16, kind="ExternalInput")
    wkT_d = nc.dram_tensor("wkT", [P, DCH * P], bf16, kind="ExternalInput")
    wvT_d = nc.dram_tensor("wvT", [P, DCH * P], bf16, kind="ExternalInput")
    # packed per-partition constants: [bq | bk | bvb(128) | maskT(8*4)]
    misc_d = nc.dram_tensor("misc", [P, 2 + P + DCH * B], f32,
                            kind="ExternalInput")
    out_d = nc.dram_tensor("out", [B, HPC, DH, S], f32, kind="ExternalOutput")

    with tile.TileContext(nc) as tc, ExitStack() as ctx:
        persist = ctx.enter_context(tc.tile_pool(name="persist", bufs=1))
        pp_pool = ctx.enter_context(tc.tile_pool(name="pp", bufs=34))
        stage = ctx.enter_context(tc.tile_pool(name="stage", bufs=8))
        ps_sc = ctx.enter_context(tc.tile_pool(name="ps_sc", bufs=2, space="PSUM"))
        ps_mm = ctx.enter_context(tc.tile_pool(name="ps_mm", bufs=2, space="PSUM"))
        ps_ctx = ctx.enter_context(tc.tile_pool(name="ps_ctx", bufs=2, space="PSUM"))

        # ---- persistent SBUF tensors ----
        hT_sb = persist.tile([P, B, DCH, S], bf16)      # hidden^T, batch/d-tiled
        wq_sb = persist.tile([P, DCH, P], bf16)
        wk_sb = persist.tile([P, DCH, P], bf16)
        wv_sb = persist.tile([P, DCH, P], bf16)
        misc_sb = persist.tile([P, 2 + P + DCH * B], f32)
        qT_sb = persist.tile([P, BS], bf16)             # Q^T [i, q]
        kT_sb = persist.tile([P, BS], bf16)             # K^T [i, u]
        # V in [u, j] layout + ones column per head: [u-part, u-tile, head, DH+1]
        v_sb = persist.tile([P, BS // P, HPC, DH + 1], bf16)
        dummy_sb = persist.tile([P, P], bf16)

        bq_sb = misc_sb[:, 0:1]
        bk_sb = misc_sb[:, 1:2]
        bvb_sb = misc_sb[:, 2:2 + P]

        def mask_bias(uc, bi):
            c = 2 + P + uc * B + bi
            return misc_sb[:, c:c + 1]

        # ---- HAM warmup: dead matmuls keep the PE busy while inputs load,
        # so the real work starts at the 2.4 GHz clock.
        nc.vector.memset(dummy_sb[:], 0.0)
        d_ps = ps_mm.tile([P, P], f32, tag="mm", name="d_ps")
        for _ in range(48):
            nc.tensor.matmul(d_ps[:], dummy_sb[:], dummy_sb[:],
                             start=True, stop=True)

        # ---- input DMAs ----
        # Queue plan: weights+misc ride the gpsimd queue (idle early);
        # batch 0 lands as two partition-half DMAs on sync/scalar (16KB
        # contiguous per partition -> near-peak bandwidth); batches 1-3 as
        # one DMA each on sync/gpsimd, keeping the scalar queue free for
        # activations (a queued DMA issue blocks the queue ~2us on
        # semaphore recycling once >4 are in flight).
        hT_re = hT_d.ap().rearrange("p (b dc t) -> p b dc t", dc=DCH, t=S)

        nc.sync.dma_start(wq_sb[:],
                          wqT_d.ap().rearrange("p (dc m) -> p dc m", m=P))
        nc.scalar.dma_start(wk_sb[:],
                            wkT_d.ap().rearrange("p (dc m) -> p dc m", m=P))
        for dc in range(DCH):
            eng = nc.sync if dc % 2 == 0 else nc.scalar
            eng.dma_start(hT_sb[:, 0, dc, :], hT_re[:, 0, dc, :])
        nc.gpsimd.dma_start(wv_sb[:],
                            wvT_d.ap().rearrange("p (dc m) -> p dc m", m=P))
        nc.gpsimd.dma_start(misc_sb[:], misc_d.ap())
        for bslot in range(1, B):
            for dc in range(DCH):
                eng = nc.sync if dc % 2 == 0 else nc.gpsimd
                eng.dma_start(hT_sb[:, bslot, dc, :], hT_re[:, bslot, dc, :])

        nc.vector.memset(v_sb[:, :, :, 0:1], 1.0)

        scale = 1.0 / float(np.sqrt(DH))

        # ---- projection thunks -------------------------------------------
        def emit_qk_chunk(w_sb, b_sb, dst, ci):
            sl = slice(ci * CH, (ci + 1) * CH)
            lsl = slice((ci % 2) * CH, (ci % 2 + 1) * CH)
            qk_ps = ps_mm.tile([P, CH], f32, tag="mm", name="qk_ps")
            for dc in range(DCH):
                nc.tensor.matmul(
                    qk_ps[:], w_sb[:, dc, :], hT_sb[:, ci // 2, dc, lsl],
                    start=(dc == 0), stop=(dc == DCH - 1))
            nc.vector.tensor_scalar_add(dst[:, sl], qk_ps[:], b_sb[:])

        def emit_v_tile(tt):
            tsl = slice((tt % DCH) * P, (tt % DCH + 1) * P)
            v_ps = ps_mm.tile([P, P], f32, tag="mm", name="v_ps")
            for dc in range(DCH):
                nc.tensor.matmul(
                    v_ps[:], hT_sb[:, tt // DCH, dc, tsl], wv_sb[:, dc, :],
                    start=(dc == 0), stop=(dc == DCH - 1))
            nc.vector.tensor_add(
                v_sb[:, tt, 0:HPC, 1:DH + 1],
                v_ps[:].rearrange("p (h j) -> p h j", j=DH),
                bvb_sb[:].rearrange("p (h j) -> p h j", j=DH))

        def qk_thunks(bi):
            th = []
            for w_sb, b_sb, dst in ((wq_sb, bq_sb, qT_sb), (wk_sb, bk_sb, kT_sb)):
                for ci in range(2 * bi, 2 * bi + 2):
                    th.append((4096, lambda w=w_sb, b=b_sb, d=dst, c=ci:
                               emit_qk_chunk(w, b, d, c)))
            return th

        def v_thunks(bi):
            return [(1024, lambda t=tt: emit_v_tile(t))
                    for tt in range(8 * bi, 8 * bi + 8)]

        # ---- AV + normalization ------------------------------------------
        def emit_norm(bi, hl, c2, ctx_ps):
            csl = slice(c2 * CH, (c2 + 1) * CH)
            rcp_sb = stage.tile([1, CH], f32, tag="rcp", bufs=2, name="rcp_sb")
            nc.vector.reciprocal_approx_fast(rcp_sb[:], ctx_ps[0:1, :])
            rcpb_sb = stage.tile([DH + 1, CH], f32, tag="rcpb", bufs=2,
                                 name="rcpb_sb")
            nc.gpsimd.partition_broadcast(rcpb_sb[:], rcp_sb[:])
            o_sb = stage.tile([DH + 1, CH], f32, tag="o", bufs=3, name="o_sb")
            nc.vector.tensor_mul(o_sb[:], ctx_ps[:], rcpb_sb[:])
            nc.sync.dma_start(out_d.ap()[bi, hl, :, csl], o_sb[1:DH + 1, :])

        def emit_av_mm(bi, hl, c2, ctx_ps, uc, pps):
            nc.tensor.matmul(
                ctx_ps[:],
                v_sb[:, bi * 8 + uc, hl, :],
                pps[(uc, hl)][:, c2 * CH:(c2 + 1) * CH],
                start=(uc == 0), stop=(uc == DCH - 1))

        # ---- prologue: batch 0's Q/K, dc-major so all four PSUM groups
        # chase the arriving hT pieces concurrently.
        pro_specs = [(wq_sb, bq_sb, qT_sb, 0), (wq_sb, bq_sb, qT_sb, 1),
                     (wk_sb, bk_sb, kT_sb, 0), (wk_sb, bk_sb, kT_sb, 1)]
        # groups 0-1 use the first CH columns of a [P, S] ps_sc tile (2 banks
        # each); groups 2-3 use [P, CH] ps_mm tiles.
        pro_tiles = [ps_sc.tile([P, S], f32, tag="sc", name="pro_ps"),
                     ps_sc.tile([P, S], f32, tag="sc", name="pro_ps"),
                     ps_mm.tile([P, CH], f32, tag="mm", name="pro_ps"),
                     ps_mm.tile([P, CH], f32, tag="mm", name="pro_ps")]
        pro_ps = [t[:, 0:CH] for t in pro_tiles]
        for dc in range(DCH):
            for g, (w_sb, b_sb, dst, ci) in enumerate(pro_specs):
                nc.tensor.matmul(
                    pro_ps[g], w_sb[:, dc, :],
                    hT_sb[:, 0, dc, ci * CH:(ci + 1) * CH],
                    start=(dc == 0), stop=(dc == DCH - 1))
        # bias-adds ordered (q-c0, k-c0 first) and split DVE/ACT so the
        # first score matmuls can start after three of the four.
        for g in (0, 2, 1, 3):
            w_sb, b_sb, dst, ci = pro_specs[g]
            osl = slice(ci * CH, (ci + 1) * CH)
            if g in (0, 1):
                nc.vector.tensor_scalar_add(dst[:, osl], pro_ps[g], b_sb[:])
            else:
                nc.scalar.activation(dst[:, osl], pro_ps[g],
                                     AF.Identity, bias=b_sb[:])

        # ---- main software pipeline over batches -------------------------
        queue = []          # (cycles, thunk) list, paced per uc
        queue += v_thunks(0) + qk_thunks(1) + v_thunks(1)
        prev_pps = None     # probs tiles of batch bi-1
        ctx_cur = None
        ctx3 = None

        for bi in range(B):
            if bi == 1:
                queue += qk_thunks(2) + v_thunks(2)
            elif bi == 2:
                queue += qk_thunks(3) + v_thunks(3)
            total_c = sum(c for c, _ in queue)
            popped = [0]

            def pace(uc, _total=total_c, _popped=popped, _queue=queue):
                want = (_total * (uc + 1)) // DCH
                while _queue and _popped[0] < want:
                    cyc, th = _queue.pop(0)
                    _popped[0] += cyc
                    th()

            pps = {}
            for uc in range(DCH):
                usl = slice(bi * S + uc * P, bi * S + (uc + 1) * P)
                # scores: both heads' chunk-c matmuls adjacent so the PE
                # row-tiles (0,0)/(64,0) run them concurrently.
                sc = [ps_sc.tile([P, S], f32, tag="sc", name="sc_ps") for _ in range(HPC)]
                for c2 in range(2):
                    csl = slice(c2 * CH, (c2 + 1) * CH)
                    qsl = slice(bi * S + c2 * CH, bi * S + (c2 + 1) * CH)
                    for hl in range(HPC):
                        hsl = slice(hl * DH, (hl + 1) * DH)
                        nc.tensor.matmul(
                            sc[hl][:, csl],
                            kT_sb[hsl, usl], qT_sb[hsl, qsl],
                            start=True, stop=True)
                for hl in range(HPC):
                    pp = pp_pool.tile([P, S], bf16, tag="pp", name="pp")
                    nc.scalar.activation(
                        pp[:], sc[hl][:], AF.Exp,
                        bias=mask_bias(uc, bi), scale=scale)
                    pps[(uc, hl)] = pp

                # AV of batch bi-1, chunk-serial: one ctx bank at a time.
                if prev_pps is not None:
                    seg = uc // 2          # 0:h0c0 1:h0c1 2:h1c0 3:h1c1
                    hl, c2 = seg // 2, seg % 2
                    half = uc % 2
                    if half == 0:
                        ctx_cur = ps_ctx.tile([DH + 1, CH], f32, tag="ctx", name="ctx_ps")
                    for up in range(4 * half, 4 * half + 4):
                        emit_av_mm(bi - 1, hl, c2, ctx_cur, up, prev_pps)
                    if half == 1:
                        emit_norm(bi - 1, hl, c2, ctx_cur)

                # batch 3's own h0 AV woven into its loop (ps_mm is idle).
                if bi == B - 1:
                    if uc == 0:
                        ctx3 = [ps_mm.tile([DH + 1, CH], f32, tag="mm", name="ctx3")
                                for _ in range(2)]
                    for c2 in range(2):
                        emit_av_mm(bi, 0, c2, ctx3[c2], uc, pps)
                    if uc == DCH - 1:
                        for c2 in range(2):
                            emit_norm(bi, 0, c2, ctx3[c2])

                pace(uc)
            prev_pps = pps

        # ---- epilogue: batch 3's h1 --------------------------------------
        ctxE = [ps_ctx.tile([DH + 1, CH], f32, tag="ctx", name="ctxE") for _ in range(2)]
        for c2 in range(2):
            for uc in range(DCH):
                emit_av_mm(B - 1, 1, c2, ctxE[c2], uc, prev_pps)
            emit_norm(B - 1, 1, c2, ctxE[c2])

    nc.compile()
    return nc


def _get_nc():
    global _compiled_nc
    if _compiled_nc is None:
        _compiled_nc = _build()
    return _compiled_nc


def prepare_in_maps(hidden_states, attention_mask, Wq, bq, Wk, bk, Wv, bv):
    bf16 = ml_dtypes.bfloat16

    hs = np.asarray(hidden_states, dtype=np.float32)            # [S, B, D]
    # [p, b, dc, t']: hT[p, b, dc, t'] = hs[t', b, dc*128+p]
    hT = np.ascontiguousarray(
        hs.reshape(S, B, DCH, P).transpose(3, 1, 2, 0)
          .reshape(P, B * DCH * S)).astype(bf16)
    maskT = np.ascontiguousarray(
        np.asarray(attention_mask, dtype=np.float32).reshape(B, S).T)
    Wq = np.asarray(Wq, dtype=np.float32)
    Wk = np.asarray(Wk, dtype=np.float32)
    Wv = np.asarray(Wv, dtype=np.float32)
    bq = np.asarray(bq, dtype=np.float32)
    bk = np.asarray(bk, dtype=np.float32)
    bv = np.asarray(bv, dtype=np.float32)

    # maskT packed as [p, uc, b] -> [128, 32]
    mask_pk = maskT.reshape(DCH, P, B).transpose(1, 0, 2).reshape(P, DCH * B)
    in_maps = []
    for c in range(NCORES):
        sl = slice(P * c, P * (c + 1))
        misc = np.empty((P, 2 + P + DCH * B), dtype=np.float32)
        misc[:, 0] = bq[sl]
        misc[:, 1] = bk[sl]
        misc[:, 2:2 + P] = np.broadcast_to(bv[sl][None, :], (P, P))
        misc[:, 2 + P:] = mask_pk
        def pack_w(W):
            # [p, dc*128+m] = W[sl][m, dc*128+p] -> contiguous upload rows
            wT = W[sl, :].T.reshape(DCH, P, P)          # [dc, p, m]
            return np.ascontiguousarray(
                wT.transpose(1, 0, 2).reshape(P, DCH * P)).astype(bf16)

        in_maps.append({
            "hT": hT,
            "wqT": pack_w(Wq),
            "wkT": pack_w(Wk),
            "wvT": pack_w(Wv),
            "misc": misc,
        })
    return in_maps


def kernel(hidden_states, attention_mask, Wq, bq, Wk, bk, Wv, bv):
    global last_exec_time_ns, last_results
    from concourse.bass_utils import run_bass_kernel_spmd

    nc = _get_nc()
    in_maps = prepare_in_maps(hidden_states, attention_mask,
                              Wq, bq, Wk, bk, Wv, bv)

    trace = bool(int(os.environ.get("KERNEL_TRACE", "0")))
    tmpdir = os.environ.get("KERNEL_TRACE_DIR") or None
    res = run_bass_kernel_spmd(nc, in_maps, core_ids=list(range(NCORES)),
                               trace=trace, tmpdir=tmpdir)
    last_exec_time_ns = res.exec_time_ns
    last_results = res

    # gather: per-core out [B, HPC, DH, S] -> full [S, B, D]
    outs = np.stack([np.asarray(res.results[c]["out"]) for c in range(NCORES)],
                    axis=0)                                     # [C, B, HPC, DH, S]
    full = outs.transpose(4, 1, 0, 2, 3).reshape(S, B, D)       # s, b, (c, hl, j)
    return np.ascontiguousarray(full.astype(np.float32))


# revision 27
# speedup vs baseline: 1.0888x; 1.0262x over previous
"""BertSelfAttention Trainium2 kernel.

Shapes: hidden_states [S=1024, B=4, D=1024], H=16 heads of DH=64.
Sharding: 2 heads per core (8 cores). Each core receives the full hidden
states (pre-transposed + bf16-cast on host) and a 128-row slice of each
projection weight, computes the full attention chain for its two heads with
no cross-core communication, and writes ctx^T per (batch, head).

Device-side layout:
  - scores are computed transposed (scoresT[u, q] = q_q . k_u) so the
    additive attention mask (per key position u) is a per-partition bias
    that fuses into the Exp activation: probsT = exp(scores/8 + mask).
  - the two heads' score matmuls have contraction 64, so they land on PE
    row-tiles (0,0)/(64,0) and run concurrently; each (uc, head) score
    tile is [128, 1024] spanning 2 PSUM banks so a single N=1024 Exp
    serves both q-chunks.
  - V carries a prepended ones-column, so the AV matmul produces the
    softmax denominator in row 0 of ctxT for free; normalization is
    reciprocal_approx_fast + gpsimd partition_broadcast + one multiply,
    done per 512-wide chunk as soon as its ctx accumulation finishes.
  - AV of batch b-1 is woven chunk-serial through batch b's uc loop
    (h0c0 in uc0-1, h0c1 in uc2-3, h1c0 in uc4-5, h1c1 in uc6-7) so only
    one ctx PSUM bank is held at a time; batch 3's own h0 AV is woven
    into its loop and only h1 remains in the epilogue.
"""

import os
import numpy as np
import ml_dtypes

S, B, D, H = 1024, 4, 1024, 16
DH = D // H          # 64
NCORES = 8
HPC = H // NCORES    # heads per core = 2
P = 128              # partitions / d-tile / u-tile
DCH = D // P         # 8 contraction tiles
BS = B * S           # 4096 flattened (b, s)
CH = 512             # matmul free-dim chunk (PSUM bank limit)

_compiled_nc = None
last_exec_time_ns = None
last_results = None


def _build():
    import concourse.bacc as bacc
    import concourse.mybir as mybir
    import concourse.tile as tile
    from contextlib import ExitStack

    f32 = mybir.dt.float32
    bf16 = mybir.dt.bfloat16
    AF = mybir.ActivationFunctionType

    nc = bacc.Bacc("TRN2", target_bir_lowering=False, debug=False,
                   num_devices=NCORES)

    # hT host-packed as [p, b, dc, t'] so each partition's per-batch slice
    # is 16KB contiguous (big DMA descriptors -> near-peak DMA bandwidth).
    hT_d = nc.dram_tensor("hT", [P, B * DCH * S], bf16, kind="ExternalInput")
    # weights host-packed to the SBUF layout [p, dc*P+m] so the upload DMA
    # reads 2KB contiguous lines per partition.
    wqT_d = nc.dram_tensor("wqT", [P, DCH * P], bf16, kind="ExternalInput")
    wkT_d = nc.dram_tensor("wkT", [P, DCH * P], bf16, kind="ExternalInput")
    wvT_d = nc.dram_tensor("wvT", [P, DCH * P], bf16, kind="ExternalInput")
    # packed per-partition constants: [bq | bk | bvb(128) | maskT(8*4)]
    misc_d = nc.dram_tensor("misc", [P, 2 + P + DCH * B], f32,
                            kind="ExternalInput")
    out_d = nc.dram_tensor("out", [B, HPC, DH, S], f32, kind="ExternalOutput")

    with tile.TileContext(nc) as tc, ExitStack() as ctx:
        persist = ctx.enter_context(tc.tile_pool(name="persist", bufs=1))
        pp_pool = ctx.enter_context(tc.tile_pool(name="pp", bufs=34))
        stage = ctx.enter_context(tc.tile_pool(name="stage", bufs=8))
        ps_sc = ctx.enter_context(tc.tile_pool(name="ps_sc", bufs=2, space="PSUM"))
        ps_mm = ctx.enter_context(tc.tile_pool(name="ps_mm", bufs=2, space="PSUM"))
        ps_ctx = ctx.enter_context(tc.tile_pool(name="ps_ctx", bufs=2, space="PSUM"))

        # ---- persistent SBUF tensors ----
        hT_sb = persist.tile([P, B, DCH, S], bf16)      # hidden^T, batch/d-tiled
        wq_sb = persist.tile([P, DCH, P], bf16)
        wk_sb = persist.tile([P, DCH, P], bf16)
        wv_sb = persist.tile([P, DCH, P], bf16)
        misc_sb = persist.tile([P, 2 + P + DCH * B], f32)
        qT_sb = persist.tile([P, BS], bf16)             # Q^T [i, q]
        kT_sb = persist.tile([P, BS], bf16)             # K^T [i, u]
        # V in [u, j] layout + ones column per head: [u-part, u-tile, head, DH+1]
        v_sb = persist.tile([P, BS // P, HPC, DH + 1], bf16)
        dummy_sb = persist.tile([P, P], bf16)

        bq_sb = misc_sb[:, 0:1]
        bk_sb = misc_sb[:, 1:2]
        bvb_sb = misc_sb[:, 2:2 + P]

        def mask_bias(uc, bi):
            c = 2 + P + uc * B + bi
            return misc_sb[:, c:c + 1]

        # ---- HAM warmup: dead matmuls keep the PE busy while inputs load,
        # so the real work starts at the 2.4 GHz clock.
        nc.vector.memset(dummy_sb[:], 0.0)
        d_ps = ps_mm.tile([P, P], f32, tag="mm", name="d_ps")
        for _ in range(48):
            nc.tensor.matmul(d_ps[:], dummy_sb[:], dummy_sb[:],
                             start=True, stop=True)

        # ---- input DMAs ----
        # Queue plan: weights+misc ride the gpsimd queue (idle early);
        # batch 0 lands as two partition-half DMAs on sync/scalar (16KB
        # contiguous per partition -> near-peak bandwidth); batches 1-3 as
        # one DMA each on sync/gpsimd, keeping the scalar queue free for
        # activations (a queued DMA issue blocks the queue ~2us on
        # semaphore recycling once >4 are in flight).
        hT_re = hT_d.ap().rearrange("p (b dc t) -> p b dc t", dc=DCH, t=S)

        nc.gpsimd.dma_start(wq_sb[:],
                            wqT_d.ap().rearrange("p (dc m) -> p dc m", m=P))
        nc.gpsimd.dma_start(wk_sb[:],
                            wkT_d.ap().rearrange("p (dc m) -> p dc m", m=P))
        for dc in range(DCH):
            eng = nc.sync if dc % 2 == 0 else nc.scalar
            eng.dma_start(hT_sb[:, 0, dc, :], hT_re[:, 0, dc, :])
        nc.gpsimd.dma_start(wv_sb[:],
                            wvT_d.ap().rearrange("p (dc m) -> p dc m", m=P))
        nc.gpsimd.dma_start(misc_sb[:], misc_d.ap())
        for bslot in range(1, B):
            for dc in range(DCH):
                eng = nc.sync if dc % 2 == 0 else nc.gpsimd
                eng.dma_start(hT_sb[:, bslot, dc, :], hT_re[:, bslot, dc, :])

        nc.vector.memset(v_sb[:, :, :, 0:1], 1.0)

        scale = 1.0 / float(np.sqrt(DH))

        # ---- projection thunks -------------------------------------------
        def emit_qk_chunk(w_sb, b_sb, dst, ci):
            sl = slice(ci * CH, (ci + 1) * CH)
            lsl = slice((ci % 2) * CH, (ci % 2 + 1) * CH)
            qk_ps = ps_mm.tile([P, CH], f32, tag="mm", name="qk_ps")
            for dc in range(DCH):
                nc.tensor.matmul(
                    qk_ps[:], w_sb[:, dc, :], hT_sb[:, ci // 2, dc, lsl],
                    start=(dc == 0), stop=(dc == DCH - 1))
            nc.vector.tensor_scalar_add(dst[:, sl], qk_ps[:], b_sb[:])

        def emit_v_tile(tt):
            tsl = slice((tt % DCH) * P, (tt % DCH + 1) * P)
            v_ps = ps_mm.tile([P, P], f32, tag="mm", name="v_ps")
            for dc in range(DCH):
                nc.tensor.matmul(
                    v_ps[:], hT_sb[:, tt // DCH, dc, tsl], wv_sb[:, dc, :],
                    start=(dc == 0), stop=(dc == DCH - 1))
            nc.vector.tensor_add(
                v_sb[:, tt, 0:HPC, 1:DH + 1],
                v_ps[:].rearrange("p (h j) -> p h j", j=DH),
                bvb_sb[:].rearrange("p (h j) -> p h j", j=DH))

        def qk_thunks(bi):
            th = []
            for w_sb, b_sb, dst in ((wq_sb, bq_sb, qT_sb), (wk_sb, bk_sb, kT_sb)):
                for ci in range(2 * bi, 2 * bi + 2):
                    th.append((4096, lambda w=w_sb, b=b_sb, d=dst, c=ci:
                               emit_qk_chunk(w, b, d, c)))
            return th

        def v_thunks(bi):
            return [(1024, lambda t=tt: emit_v_tile(t))
                    for tt in range(8 * bi, 8 * bi + 8)]

        # ---- AV + normalization ------------------------------------------
        def emit_norm(bi, hl, c2, ctx_ps):
            csl = slice(c2 * CH, (c2 + 1) * CH)
            rcp_sb = stage.tile([1, CH], f32, tag="rcp", bufs=2, name="rcp_sb")
            nc.vector.reciprocal_approx_fast(rcp_sb[:], ctx_ps[0:1, :])
            rcpb_sb = stage.tile([DH + 1, CH], f32, tag="rcpb", bufs=2,
                                 name="rcpb_sb")
            nc.gpsimd.partition_broadcast(rcpb_sb[:], rcp_sb[:])
            o_sb = stage.tile([DH + 1, CH], f32, tag="o", bufs=3, name="o_sb")
            nc.vector.tensor_mul(o_sb[:], ctx_ps[:], rcpb_sb[:])
            nc.sync.dma_start(out_d.ap()[bi, hl, :, csl], o_sb[1:DH + 1, :])

        def emit_av_mm(bi, hl, c2, ctx_ps, uc, pps):
            nc.tensor.matmul(
                ctx_ps[:],
                v_sb[:, bi * 8 + uc, hl, :],
                pps[(uc, hl)][:, c2 * CH:(c2 + 1) * CH],
                start=(uc == 0), stop=(uc == DCH - 1))

        # ---- prologue: batch 0's Q/K, dc-major so all four PSUM groups
        # chase the arriving hT pieces concurrently.
        pro_specs = [(wq_sb, bq_sb, qT_sb, 0), (wq_sb, bq_sb, qT_sb, 1),
                     (wk_sb, bk_sb, kT_sb, 0), (wk_sb, bk_sb, kT_sb, 1)]
        # groups 0-1 use the first CH columns of a [P, S] ps_sc tile (2 banks
        # each); groups 2-3 use [P, CH] ps_mm tiles.
        pro_tiles = [ps_sc.tile([P, S], f32, tag="sc", name="pro_ps"),
                     ps_sc.tile([P, S], f32, tag="sc", name="pro_ps"),
                     ps_mm.tile([P, CH], f32, tag="mm", name="pro_ps"),
                     ps_mm.tile([P, CH], f32, tag="mm", name="pro_ps")]
        pro_ps = [t[:, 0:CH] for t in pro_tiles]
        for dc in range(DCH):
            for g, (w_sb, b_sb, dst, ci) in enumerate(pro_specs):
                nc.tensor.matmul(
                    pro_ps[g], w_sb[:, dc, :],
                    hT_sb[:, 0, dc, ci * CH:(ci + 1) * CH],
                    start=(dc == 0), stop=(dc == DCH - 1))
        for g, (w_sb, b_sb, dst, ci) in enumerate(pro_specs):
            osl = slice(ci * CH, (ci + 1) * CH)
            nc.vector.tensor_scalar_add(dst[:, osl], pro_ps[g], b_sb[:])

        # ---- main software pipeline over batches -------------------------
        queue = []          # (cycles, thunk) list, paced per uc
        queue += v_thunks(0) + qk_thunks(1) + v_thunks(1)
        prev_pps = None     # probs tiles of batch bi-1
        ctx_cur = None
        ctx3 = None

        for bi in range(B):
            if bi == 1:
                queue += qk_thunks(2) + v_thunks(2)
            elif bi == 2:
                queue += qk_thunks(3) + v_thunks(3)
            total_c = sum(c for c, _ in queue)
            popped = [0]

            def pace(uc, _total=total_c, _popped=popped, _queue=queue):
                want = (_total * (uc + 1)) // DCH
                while _queue and _popped[0] < want:
                    cyc, th = _queue.pop(0)
                    _popped[0] += cyc
                    th()

            pps = {}
            for uc in range(DCH):
                usl = slice(bi * S + uc * P, bi * S + (uc + 1) * P)
                # scores: both heads' chunk-c matmuls adjacent so the PE
                # row-tiles (0,0)/(64,0) run them concurrently.
                sc = [ps_sc.tile([P, S], f32, tag="sc", name="sc_ps") for _ in range(HPC)]
                for c2 in range(2):
                    csl = slice(c2 * CH, (c2 + 1) * CH)
                    qsl = slice(bi * S + c2 * CH, bi * S + (c2 + 1) * CH)
                    for hl in range(HPC):
                        hsl = slice(hl * DH, (hl + 1) * DH)
                        nc.tensor.matmul(
                            sc[hl][:, csl],
                            kT_sb[hsl, usl], qT_sb[hsl, qsl],
                            start=True, stop=True)
                for hl in range(HPC):
                    pp = pp_pool.tile([P, S], bf16, tag="pp", name="pp")
                    nc.scalar.activation(
                        pp[:], sc[hl][:], AF.Exp,
                        bias=mask_bias(uc, bi), scale=scale)
                    pps[(uc, hl)] = pp

                # AV of batch bi-1, chunk-serial: one ctx bank at a time.
                if prev_pps is not None:
                    seg = uc // 2          # 0:h0c0 1:h0c1 2:h1c0 3:h1c1
                    hl, c2 = seg // 2, seg % 2
                    half = uc % 2
                    if half == 0:
                        ctx_cur = ps_ctx.tile([DH + 1, CH], f32, tag="ctx", name="ctx_ps")
                    for up in range(4 * half, 4 * half + 4):
                        emit_av_mm(bi - 1, hl, c2, ctx_cur, up, prev_pps)
                    if half == 1:
                        emit_norm(bi - 1, hl, c2, ctx_cur)

                # batch 3's own h0 AV woven into its loop (ps_mm is idle).
                if bi == B - 1:
                    if uc == 0:
                        ctx3 = [ps_mm.tile([DH + 1, CH], f32, tag="mm", name="ctx3")
                                for _ in range(2)]
                    for c2 in range(2):
                        emit_av_mm(bi, 0, c2, ctx3[c2], uc, pps)
                    if uc == DCH - 1:
                        for c2 in range(2):
                            emit_norm(bi, 0, c2, ctx3[c2])

                pace(uc)
            prev_pps = pps

        # ---- epilogue: batch 3's h1 --------------------------------------
        ctxE = [ps_ctx.tile([DH + 1, CH], f32, tag="ctx", name="ctxE") for _ in range(2)]
        for c2 in range(2):
            for uc in range(DCH):
                emit_av_mm(B - 1, 1, c2, ctxE[c2], uc, prev_pps)
            emit_norm(B - 1, 1, c2, ctxE[c2])

    nc.compile()
    return nc


def _get_nc():
    global _compiled_nc
    if _compiled_nc is None:
        _compiled_nc = _build()
    return _compiled_nc


def prepare_in_maps(hidden_states, attention_mask, Wq, bq, Wk, bk, Wv, bv):
    bf16 = ml_dtypes.bfloat16

    hs = np.asarray(hidden_states, dtype=np.float32)            # [S, B, D]
    # [p, b, dc, t']: hT[p, b, dc, t'] = hs[t', b, dc*128+p]
    hT = np.ascontiguousarray(
        hs.reshape(S, B, DCH, P).transpose(3, 1, 2, 0)
          .reshape(P, B * DCH * S)).astype(bf16)
    maskT = np.ascontiguousarray(
        np.asarray(attention_mask, dtype=np.float32).reshape(B, S).T)
    Wq = np.asarray(Wq, dtype=np.float32)
    Wk = np.asarray(Wk, dtype=np.float32)
    Wv = np.asarray(Wv, dtype=np.float32)
    bq = np.asarray(bq, dtype=np.float32)
    bk = np.asarray(bk, dtype=np.float32)
    bv = np.asarray(bv, dtype=np.float32)

    # maskT packed as [p, uc, b] -> [128, 32]
    mask_pk = maskT.reshape(DCH, P, B).transpose(1, 0, 2).reshape(P, DCH * B)
    in_maps = []
    for c in range(NCORES):
        sl = slice(P * c, P * (c + 1))
        misc = np.empty((P, 2 + P + DCH * B), dtype=np.float32)
        misc[:, 0] = bq[sl]
        misc[:, 1] = bk[sl]
        misc[:, 2:2 + P] = np.broadcast_to(bv[sl][None, :], (P, P))
        misc[:, 2 + P:] = mask_pk
        def pack_w(W):
            # [p, dc*128+m] = W[sl][m, dc*128+p] -> contiguous upload rows
            wT = W[sl, :].T.reshape(DCH, P, P)          # [dc, p, m]
            return np.ascontiguousarray(
                wT.transpose(1, 0, 2).reshape(P, DCH * P)).astype(bf16)

        in_maps.append({
            "hT": hT,
            "wqT": pack_w(Wq),
            "wkT": pack_w(Wk),
            "wvT": pack_w(Wv),
            "misc": misc,
        })
    return in_maps


def kernel(hidden_states, attention_mask, Wq, bq, Wk, bk, Wv, bv):
    global last_exec_time_ns, last_results
    from concourse.bass_utils import run_bass_kernel_spmd

    nc = _get_nc()
    in_maps = prepare_in_maps(hidden_states, attention_mask,
                              Wq, bq, Wk, bk, Wv, bv)

    trace = bool(int(os.environ.get("KERNEL_TRACE", "0")))
    tmpdir = os.environ.get("KERNEL_TRACE_DIR") or None
    res = run_bass_kernel_spmd(nc, in_maps, core_ids=list(range(NCORES)),
                               trace=trace, tmpdir=tmpdir)
    last_exec_time_ns = res.exec_time_ns
    last_results = res

    # gather: per-core out [B, HPC, DH, S] -> full [S, B, D]
    outs = np.stack([np.asarray(res.results[c]["out"]) for c in range(NCORES)],
                    axis=0)                                     # [C, B, HPC, DH, S]
    full = outs.transpose(4, 1, 0, 2, 3).reshape(S, B, D)       # s, b, (c, hl, j)
    return np.ascontiguousarray(full.astype(np.float32))
